# revision 14
# baseline (speedup 1.0000x reference)
"""Trainium2 Bass kernel for CropConv: 3x3 same-padding conv (64->64 ch) on
[16, 64, 128, 128] fp32 input, with a static crop mask zeroing output rows/cols
[44:84).

Strategy (data-parallel over batch, 8 cores x 2 images each):
  - Host marshals x into a zero-padded row-major layout with row stride 129
    (131 padded rows), so every conv tap (kh, kw) of an output row-chunk is one
    contiguous rhs slice.
  - Per core, image 0 lives in SBUF partitions 0-63 (partition = in-channel),
    image 1 in partitions 64-127.
  - The conv is 9 PSUM-accumulated TensorE matmuls per output chunk; four
    64x64-quadrant matmuls run concurrently (row-half = image, col-half =
    chunk pairing (c, c+22)).
  - x is loaded in 6 segments interleaved lower/upper band in consumption
    order so the first matmul can start early.
  - PSUM -> SBUF evictions are single fused 128-partition copies (2 per pair,
    fp32 -> fp16 cast) so the eviction engines have ample slack vs the matmul
    cadence and never stall the TensorE (which would trip the HAM clock gate).
  - Output staged in fp16; fine-grained 9-row stores stream out in order on
    the sync DMA queue as soon as rows complete; host upcasts to fp32 and
    zeroes the crop window (no device memsets).
"""

import numpy as np

# ---- problem constants (hardcoded; kernel.py must be self-contained) ----
B, C, H, W = 16, 64, 128, 128
OC, KS = 64, 3
N_CORES = 8
IMGS = B // N_CORES  # 2 images per core

WP = W + 1            # padded row stride: 129
HP = H + 3            # padded rows in the x buffer: 131
XLEN = HP * WP        # 16899 fp32 per partition

RPC = 3               # output rows per chunk
NCH = (H + RPC - 1) // RPC          # 43 chunks per image (last has 2 rows)
NPAIR = 21            # chunk pairs (c, c+22); chunk 21 is the leftover
CHN = RPC * WP        # matmul free dim per full chunk: 387
STLEN = 2 * 66 * W    # stage free size: 16896 (= 132 rows * 128)

CROP0, CROP1 = 44, 84  # masked rows/cols [44, 84)

_CACHE = {}


def _build_module():
    import concourse.tile as tile
    from concourse import bacc, mybir

    f32 = mybir.dt.float32
    f16 = mybir.dt.float16
    bf16 = mybir.dt.bfloat16

    nc = bacc.Bacc("TRN2", target_bir_lowering=False, debug=False,
                   num_devices=N_CORES)

    x_ap = nc.dram_tensor("xin", [IMGS, C, XLEN], bf16,
                          kind="ExternalInput").ap()
    w_ap = nc.dram_tensor("wt", [C, KS * KS, OC], bf16,
                          kind="ExternalInput").ap()
    y_ap = nc.dram_tensor("yout", [IMGS, OC, H, W], f16,
                          kind="ExternalOutput").ap()

    x_bc = x_ap.rearrange("b c l -> (b c) l")  # [128, XLEN]

    with tile.TileContext(nc) as tc:
        with tc.tile_pool(name="big", bufs=1) as big, \
             tc.tile_pool(name="psum", bufs=8, space="PSUM") as pp:

            x_sb = big.tile([128, XLEN], bf16, tag="xbuf")
            stage = big.tile([128, STLEN], f16, tag="stage")
            w_sb = big.tile([128, KS * KS * OC], bf16, tag="wbuf")

            st3 = stage.rearrange("p (h w) -> p h w", w=W)    # [128, 132, 128]
            st4 = stage.rearrange("p (i h w) -> p i h w", i=2, w=W)

            # warm-up scratch for HAM (PE clock gate): dummy matmuls on zeros
            # keep the PE busy while x streams in, so real matmuls start at
            # the full 2.4 GHz clock instead of the cold 1.2 GHz.
            scr = big.tile([128, 512], bf16, tag="scratch")
            nc.vector.memset(scr[:, :], 0.0)

            # weights + x loads split across BOTH hardware DMA queues (sync +
            # scalar) so the first segments of the lower band (rows 0-65) and
            # upper band (rows 66-130) land concurrently; descriptor-issue
            # (~0.6us per dma_start) is also halved per queue.
            w_flat = w_ap.rearrange("i t o -> i (t o)")
            nc.sync.dma_start(out=w_sb[0:64, :], in_=w_flat)
            nc.scalar.dma_start(out=w_sb[64:128, :], in_=w_flat)
            for (a, b_) in [(0, 8), (8, 18), (18, 32), (32, 48), (48, 66)]:
                nc.sync.dma_start(out=x_sb[:, a * WP:b_ * WP],
                                  in_=x_bc[:, a * WP:b_ * WP])
            for (a, b_) in [(66, 74), (74, 84), (84, 98), (98, 114),
                            (114, 131)]:
                nc.scalar.dma_start(out=x_sb[:, a * WP:b_ * WP],
                                    in_=x_bc[:, a * WP:b_ * WP])

            def lhsT(half, t):
                return w_sb[half * 64:(half + 1) * 64, t * OC:(t + 1) * OC]

            def rhs(half, c, kh, kw, n):
                off = (RPC * c + kh) * WP + kw
                return x_sb[half * 64:(half + 1) * 64, off:off + n]

            TAPS = [(kh, kw) for kh in range(KS) for kw in range(KS)]

            # HAM warm-up: ~6 groups of 4-quadrant dummy matmuls (~2us cold)
            # issued before the real pairs; they only depend on the scratch
            # memset, so they run while the first x segments stream in.
            pdum1 = pp.tile([128, 512], f32, tag="ps")
            pdum2 = pp.tile([128, 512], f32, tag="ps")
            NDUM = 18
            for g in range(NDUM):
                st, sp = (g == 0), (g == NDUM - 1)
                for (pd_t, half) in ((pdum1, 0), (pdum2, 1)):
                    h0 = half * 64
                    lw = scr[h0:h0 + 64, 0:64]
                    rw = scr[h0:h0 + 64, 64:64 + CHN]
                    nc.tensor.matmul(pd_t[0:64, 0:CHN], lw, rw, start=st,
                                     stop=sp, skip_group_check=True)
                    nc.tensor.matmul(pd_t[64:128, 0:CHN], lw, rw, start=st,
                                     stop=sp, skip_group_check=True)

            def store(half, r0, nr, eng=None):
                # view rows [r0, r0+nr) of partition half -> y rows
                # 66*half + r0 ... for both images
                src = st4[half * 64:(half + 1) * 64, :, r0:r0 + nr, :]
                yr0 = 66 * half + r0
                dst = y_ap[:, :, yr0:yr0 + nr, :].rearrange(
                    "b o h w -> o b h w")
                (eng or nc.sync).dma_start(out=dst, in_=src)

            def do_pair(c):
                c2 = c + 22
                n2 = 2 * WP if c2 == NCH - 1 else CHN
                pa = pp.tile([128, 512], f32, tag="ps")
                pb = pp.tile([128, 512], f32, tag="ps")
                for t, (kh, kw) in enumerate(TAPS):
                    st, sp = (t == 0), (t == len(TAPS) - 1)
                    # img0 chunk c -> A[0:64];  img0 chunk c+22 -> A[64:128]
                    nc.tensor.matmul(pa[0:64, 0:CHN], lhsT(0, t),
                                     rhs(0, c, kh, kw, CHN), start=st, stop=sp,
                                     skip_group_check=True)
                    nc.tensor.matmul(pa[64:128, 0:n2], lhsT(0, t),
                                     rhs(0, c2, kh, kw, n2), start=st, stop=sp,
                                     skip_group_check=True)
                    # img1 chunk c -> B[0:64];  img1 chunk c+22 -> B[64:128]
                    nc.tensor.matmul(pb[0:64, 0:CHN], lhsT(1, t),
                                     rhs(1, c, kh, kw, CHN), start=st, stop=sp,
                                     skip_group_check=True)
                    nc.tensor.matmul(pb[64:128, 0:n2], lhsT(1, t),
                                     rhs(1, c2, kh, kw, n2), start=st, stop=sp,
                                     skip_group_check=True)

                # evict PSUM -> stage: one fused 128-partition copy per bank
                # (lower-half rows at free offset 3c, upper-half rows at the
                # same free offset on partitions 64-127).
                pa3 = pa[:, 0:CHN].rearrange("p (h w) -> p h w", w=WP)
                pb3 = pb[:, 0:CHN].rearrange("p (h w) -> p h w", w=WP)
                if c < NPAIR - 1:
                    nc.any.tensor_copy(st3[:, 3 * c:3 * c + 3, :],
                                       pa3[:, 0:3, 0:W])
                    nc.any.tensor_copy(st3[:, 66 + 3 * c:66 + 3 * c + 3, :],
                                       pb3[:, 0:3, 0:W])
                else:
                    # chunk 42 has only 2 rows on the upper half
                    nc.any.tensor_copy(st3[:, 60:62, :], pa3[:, 0:2, 0:W])
                    nc.any.tensor_copy(st3[0:64, 62:63, :],
                                       pa3[0:64, 2:3, 0:W])
                    nc.any.tensor_copy(st3[:, 126:128, :], pb3[:, 0:2, 0:W])
                    nc.any.tensor_copy(st3[0:64, 128:129, :],
                                       pb3[0:64, 2:3, 0:W])

            def do_leftover():
                # chunk 21 (lower rows 63-65), both images, via two banks
                pc_ = pp.tile([128, 512], f32, tag="ps")
                pd_ = pp.tile([128, 512], f32, tag="ps")
                for t, (kh, kw) in enumerate(TAPS):
                    st, sp = (t == 0), (t == len(TAPS) - 1)
                    nc.tensor.matmul(pc_[0:64, 0:CHN], lhsT(0, t),
                                     rhs(0, 21, kh, kw, CHN), start=st,
                                     stop=sp, skip_group_check=True)
                    nc.tensor.matmul(pd_[0:64, 0:CHN], lhsT(1, t),
                                     rhs(1, 21, kh, kw, CHN), start=st,
                                     stop=sp, skip_group_check=True)
                pc3 = pc_[:, 0:CHN].rearrange("p (h w) -> p h w", w=WP)
                pd3 = pd_[:, 0:CHN].rearrange("p (h w) -> p h w", w=WP)
                nc.any.tensor_copy(st3[0:64, 63:66, :], pc3[0:64, 0:3, 0:W])
                nc.any.tensor_copy(st3[0:64, 129:132, :], pd3[0:64, 0:3, 0:W])

            # stores fire every 3 pairs (9 output rows per half per image);
            # the leftover chunk runs mid-stream so only pair 20's small
            # stores remain after the last full pair.
            for c in range(16):
                do_pair(c)
                if c >= 2 and (c - 2) % 3 == 0:
                    k = (c - 2) // 3
                    store(0, 9 * k, 9)
                    store(1, 9 * k, 9)
            do_leftover()
            store(0, 63, 3)
            for c in range(16, NPAIR):
                do_pair(c)
                if c == 17:
                    store(0, 45, 9)
                    store(1, 45, 9)
                elif c >= 18:
                    # per-pair stores at the tail, split across both DMA
                    # queues, so the last transfer after the final matmul
                    # is tiny and issues in parallel
                    r0 = 54 + 3 * (c - 18)
                    store(0, r0, 3)
                    store(1, r0, 3 if c < 20 else 2, eng=nc.scalar)

    nc.compile()
    return nc


def _get_module():
    if "nc" not in _CACHE:
        _CACHE["nc"] = _build_module()
    return _CACHE["nc"]


def _make_in_maps(x, weight):
    x = np.asarray(x, dtype=np.float32)
    weight = np.asarray(weight, dtype=np.float32)
    # host marshaling: pad x into the row-major stride-129 layout
    xp = np.zeros((B, C, HP, WP), dtype=np.float32)
    xp[:, :, 1:H + 1, 1:W + 1] = x
    xp = xp.reshape(B, C, XLEN)
    import ml_dtypes
    xp = xp.astype(ml_dtypes.bfloat16)
    # weight [oc, ic, kh, kw] -> [ic, (kh kw), oc]
    wt = np.ascontiguousarray(
        weight.transpose(1, 2, 3, 0).reshape(C, KS * KS, OC)
    ).astype(ml_dtypes.bfloat16)
    return [
        {"xin": np.ascontiguousarray(xp[k * IMGS:(k + 1) * IMGS]), "wt": wt}
        for k in range(N_CORES)
    ]


def kernel(x, weight):
    from concourse.bass_utils import run_bass_kernel_spmd

    nc = _get_module()
    in_maps = _make_in_maps(x, weight)
    res = run_bass_kernel_spmd(nc, in_maps, list(range(N_CORES)))
    out = np.concatenate([res.results[k]["yout"] for k in range(N_CORES)],
                         axis=0).astype(np.float32)
    # crop mask applied on host (device never memsets the window)
    out[:, :, CROP0:CROP1, CROP0:CROP1] = 0.0
    return out


# revision 16
# speedup vs baseline: 1.1830x; 1.1830x over previous
"""Trainium2 Bass kernel for CropConv: 3x3 same-padding conv (64->64 ch) on
[16, 64, 128, 128] fp32 input, with a static crop mask zeroing output rows/cols
[44:84).

Strategy (data-parallel over batch, 8 cores x 2 images each):
  - Host marshals x into a zero-padded row-major layout with row stride 129
    (131 padded rows), so every conv tap (kh, kw) of an output row-chunk is one
    contiguous rhs slice.
  - Per core, image 0 lives in SBUF partitions 0-63 (partition = in-channel),
    image 1 in partitions 64-127.
  - The conv is 9 PSUM-accumulated TensorE matmuls per output chunk; four
    64x64-quadrant matmuls run concurrently (row-half = image, col-half =
    chunk pairing (c, c+22)).
  - x is loaded in 10 segments split across both hardware DMA queues
    (sync + scalar), lower/upper band in consumption order, so the first
    matmul can start early; dummy matmuls on a zeroed scratch warm the PE
    clock gate (HAM) while the first segments stream in.
  - PSUM -> SBUF evictions are single fused 128-partition copies (2 per pair,
    fp32 -> fp16 cast) so the eviction engines have ample slack vs the matmul
    cadence and never stall the TensorE (which would trip the HAM clock gate).
  - Output staged in fp16; fine-grained 9-row stores stream out in order on
    the sync DMA queue as soon as rows complete; host upcasts to fp32 and
    zeroes the crop window (no device memsets).
"""

import numpy as np

# ---- problem constants (hardcoded; kernel.py must be self-contained) ----
B, C, H, W = 16, 64, 128, 128
OC, KS = 64, 3
N_CORES = 8
IMGS = B // N_CORES  # 2 images per core

WP = W + 1            # padded row stride: 129
HP = H + 3            # padded rows in the x buffer: 131
XLEN = HP * WP        # 16899 fp32 per partition

RPC = 3               # output rows per chunk
NCH = (H + RPC - 1) // RPC          # 43 chunks per image (last has 2 rows)
NPAIR = 21            # chunk pairs (c, c+22); chunk 21 is the leftover
CHN = RPC * WP        # matmul free dim per full chunk: 387
STLEN = 2 * 66 * W    # stage free size: 16896 (= 132 rows * 128)

CROP0, CROP1 = 44, 84  # masked rows/cols [44, 84)

_CACHE = {}


def _build_module():
    import concourse.tile as tile
    from concourse import bacc, mybir

    f32 = mybir.dt.float32
    f16 = mybir.dt.float16
    bf16 = mybir.dt.bfloat16

    nc = bacc.Bacc("TRN2", target_bir_lowering=False, debug=False,
                   num_devices=N_CORES)

    x_ap = nc.dram_tensor("xin", [IMGS, C, XLEN], bf16,
                          kind="ExternalInput").ap()
    w_ap = nc.dram_tensor("wt", [C, KS * KS, OC], bf16,
                          kind="ExternalInput").ap()
    y_ap = nc.dram_tensor("yout", [IMGS, OC, H, W], f16,
                          kind="ExternalOutput").ap()

    x_bc = x_ap.rearrange("b c l -> (b c) l")  # [128, XLEN]

    with tile.TileContext(nc) as tc:
        with tc.tile_pool(name="big", bufs=1) as big, \
             tc.tile_pool(name="psum", bufs=8, space="PSUM") as pp:

            x_sb = big.tile([128, XLEN], bf16, tag="xbuf")
            stage = big.tile([128, STLEN], f16, tag="stage")
            w_sb = big.tile([128, KS * KS * OC], bf16, tag="wbuf")

            st3 = stage.rearrange("p (h w) -> p h w", w=W)    # [128, 132, 128]
            st4 = stage.rearrange("p (i h w) -> p i h w", i=2, w=W)

            # warm-up scratch for HAM (PE clock gate): dummy matmuls on zeros
            # keep the PE busy while x streams in, so real matmuls start at
            # the full 2.4 GHz clock instead of the cold 1.2 GHz.
            scr = big.tile([128, 512], bf16, tag="scratch")
            nc.vector.memset(scr[:, :], 0.0)

            # weights + x loads split across BOTH hardware DMA queues (sync +
            # scalar) so the first segments of the lower band (rows 0-65) and
            # upper band (rows 66-130) land concurrently; descriptor-issue
            # (~0.6us per dma_start) is also halved per queue.
            w_flat = w_ap.rearrange("i t o -> i (t o)")
            nc.sync.dma_start(out=w_sb[0:64, :], in_=w_flat)
            nc.scalar.dma_start(out=w_sb[64:128, :], in_=w_flat)
            for (a, b_) in [(0, 8), (8, 18), (18, 32), (32, 48), (48, 66)]:
                nc.sync.dma_start(out=x_sb[:, a * WP:b_ * WP],
                                  in_=x_bc[:, a * WP:b_ * WP])
            for (a, b_) in [(66, 74), (74, 84), (84, 98), (98, 114),
                            (114, 131)]:
                nc.scalar.dma_start(out=x_sb[:, a * WP:b_ * WP],
                                    in_=x_bc[:, a * WP:b_ * WP])

            def lhsT(half, t):
                return w_sb[half * 64:(half + 1) * 64, t * OC:(t + 1) * OC]

            def rhs(half, c, kh, kw, n):
                off = (RPC * c + kh) * WP + kw
                return x_sb[half * 64:(half + 1) * 64, off:off + n]

            TAPS = [(kh, kw) for kh in range(KS) for kw in range(KS)]

            # HAM warm-up: ~6 groups of 4-quadrant dummy matmuls (~2us cold)
            # issued before the real pairs; they only depend on the scratch
            # memset, so they run while the first x segments stream in.
            pdum1 = pp.tile([128, 512], f32, tag="ps")
            pdum2 = pp.tile([128, 512], f32, tag="ps")
            NDUM = 11
            for g in range(NDUM):
                st, sp = (g == 0), (g == NDUM - 1)
                for (pd_t, half) in ((pdum1, 0), (pdum2, 1)):
                    h0 = half * 64
                    lw = scr[h0:h0 + 64, 0:64]
                    rw = scr[h0:h0 + 64, 64:64 + CHN]
                    nc.tensor.matmul(pd_t[0:64, 0:CHN], lw, rw, start=st,
                                     stop=sp, skip_group_check=True)
                    nc.tensor.matmul(pd_t[64:128, 0:CHN], lw, rw, start=st,
                                     stop=sp, skip_group_check=True)

            def store(half, r0, nr, eng=None):
                # view rows [r0, r0+nr) of partition half -> y rows
                # 66*half + r0 ... for both images
                src = st4[half * 64:(half + 1) * 64, :, r0:r0 + nr, :]
                yr0 = 66 * half + r0
                dst = y_ap[:, :, yr0:yr0 + nr, :].rearrange(
                    "b o h w -> o b h w")
                (eng or nc.sync).dma_start(out=dst, in_=src)

            def do_pair(c):
                c2 = c + 22
                n2 = 2 * WP if c2 == NCH - 1 else CHN
                pa = pp.tile([128, 512], f32, tag="ps")
                pb = pp.tile([128, 512], f32, tag="ps")
                for t, (kh, kw) in enumerate(TAPS):
                    st, sp = (t == 0), (t == len(TAPS) - 1)
                    # img0 chunk c -> A[0:64];  img0 chunk c+22 -> A[64:128]
                    nc.tensor.matmul(pa[0:64, 0:CHN], lhsT(0, t),
                                     rhs(0, c, kh, kw, CHN), start=st, stop=sp,
                                     skip_group_check=True)
                    nc.tensor.matmul(pa[64:128, 0:n2], lhsT(0, t),
                                     rhs(0, c2, kh, kw, n2), start=st, stop=sp,
                                     skip_group_check=True)
                    # img1 chunk c -> B[0:64];  img1 chunk c+22 -> B[64:128]
                    nc.tensor.matmul(pb[0:64, 0:CHN], lhsT(1, t),
                                     rhs(1, c, kh, kw, CHN), start=st, stop=sp,
                                     skip_group_check=True)
                    nc.tensor.matmul(pb[64:128, 0:n2], lhsT(1, t),
                                     rhs(1, c2, kh, kw, n2), start=st, stop=sp,
                                     skip_group_check=True)

                # evict PSUM -> stage: one fused 128-partition copy per bank
                # (lower-half rows at free offset 3c, upper-half rows at the
                # same free offset on partitions 64-127).
                pa3 = pa[:, 0:CHN].rearrange("p (h w) -> p h w", w=WP)
                pb3 = pb[:, 0:CHN].rearrange("p (h w) -> p h w", w=WP)
                if c < NPAIR - 1:
                    nc.any.tensor_copy(st3[:, 3 * c:3 * c + 3, :],
                                       pa3[:, 0:3, 0:W])
                    nc.any.tensor_copy(st3[:, 66 + 3 * c:66 + 3 * c + 3, :],
                                       pb3[:, 0:3, 0:W])
                else:
                    # chunk 42 has only 2 rows on the upper half
                    nc.any.tensor_copy(st3[:, 60:62, :], pa3[:, 0:2, 0:W])
                    nc.any.tensor_copy(st3[0:64, 62:63, :],
                                       pa3[0:64, 2:3, 0:W])
                    nc.any.tensor_copy(st3[:, 126:128, :], pb3[:, 0:2, 0:W])
                    nc.any.tensor_copy(st3[0:64, 128:129, :],
                                       pb3[0:64, 2:3, 0:W])

            def do_leftover():
                # chunk 21 (lower rows 63-65), both images, via two banks
                pc_ = pp.tile([128, 512], f32, tag="ps")
                pd_ = pp.tile([128, 512], f32, tag="ps")
                for t, (kh, kw) in enumerate(TAPS):
                    st, sp = (t == 0), (t == len(TAPS) - 1)
                    nc.tensor.matmul(pc_[0:64, 0:CHN], lhsT(0, t),
                                     rhs(0, 21, kh, kw, CHN), start=st,
                                     stop=sp, skip_group_check=True)
                    nc.tensor.matmul(pd_[0:64, 0:CHN], lhsT(1, t),
                                     rhs(1, 21, kh, kw, CHN), start=st,
                                     stop=sp, skip_group_check=True)
                pc3 = pc_[:, 0:CHN].rearrange("p (h w) -> p h w", w=WP)
                pd3 = pd_[:, 0:CHN].rearrange("p (h w) -> p h w", w=WP)
                nc.any.tensor_copy(st3[0:64, 63:66, :], pc3[0:64, 0:3, 0:W])
                nc.any.tensor_copy(st3[0:64, 129:132, :], pd3[0:64, 0:3, 0:W])

            # stores fire every 3 pairs (9 output rows per half per image);
            # the leftover chunk runs mid-stream so only pair 20's small
            # stores remain after the last full pair.
            for c in range(16):
                do_pair(c)
                if c >= 2 and (c - 2) % 3 == 0:
                    k = (c - 2) // 3
                    store(0, 9 * k, 9)
                    store(1, 9 * k, 9)
            do_leftover()
            store(0, 63, 3)
            for c in range(16, NPAIR):
                do_pair(c)
                if c == 17:
                    store(0, 45, 9)
                    store(1, 45, 9)
                elif c >= 18:
                    # per-pair stores at the tail, split across both DMA
                    # queues, so the last transfer after the final matmul
                    # is tiny and issues in parallel
                    r0 = 54 + 3 * (c - 18)
                    store(0, r0, 3)
                    store(1, r0, 3 if c < 20 else 2, eng=nc.scalar)

    nc.compile()
    return nc


def _get_module():
    if "nc" not in _CACHE:
        _CACHE["nc"] = _build_module()
    return _CACHE["nc"]


def _make_in_maps(x, weight):
    x = np.asarray(x, dtype=np.float32)
    weight = np.asarray(weight, dtype=np.float32)
    # host marshaling: pad x into the row-major stride-129 layout
    xp = np.zeros((B, C, HP, WP), dtype=np.float32)
    xp[:, :, 1:H + 1, 1:W + 1] = x
    xp = xp.reshape(B, C, XLEN)
    import ml_dtypes
    xp = xp.astype(ml_dtypes.bfloat16)
    # weight [oc, ic, kh, kw] -> [ic, (kh kw), oc]
    wt = np.ascontiguousarray(
        weight.transpose(1, 2, 3, 0).reshape(C, KS * KS, OC)
    ).astype(ml_dtypes.bfloat16)
    return [
        {"xin": np.ascontiguousarray(xp[k * IMGS:(k + 1) * IMGS]), "wt": wt}
        for k in range(N_CORES)
    ]


def kernel(x, weight):
    from concourse.bass_utils import run_bass_kernel_spmd

    nc = _get_module()
    in_maps = _make_in_maps(x, weight)
    res = run_bass_kernel_spmd(nc, in_maps, list(range(N_CORES)))
    out = np.concatenate([res.results[k]["yout"] for k in range(N_CORES)],
                         axis=0).astype(np.float32)
    # crop mask applied on host (device never memsets the window)
    out[:, :, CROP0:CROP1, CROP0:CROP1] = 0.0
    return out


# revision 18
# speedup vs baseline: 1.1836x; 1.0005x over previous
"""Trainium2 Bass kernel for CropConv: 3x3 same-padding conv (64->64 ch) on
[16, 64, 128, 128] fp32 input, with a static crop mask zeroing output rows/cols
[44:84).

Strategy (data-parallel over batch, 8 cores x 2 images each):
  - Host marshals x into a zero-padded row-major layout with row stride 129
    (131 padded rows), so every conv tap (kh, kw) of an output row-chunk is one
    contiguous rhs slice.
  - Per core, image 0 lives in SBUF partitions 0-63 (partition = in-channel),
    image 1 in partitions 64-127.
  - The conv is 9 PSUM-accumulated TensorE matmuls per output chunk; four
    64x64-quadrant matmuls run concurrently (row-half = image, col-half =
    chunk pairing (c, c+22)).
  - x is loaded in 10 segments split across both hardware DMA queues
    (sync + scalar), lower/upper band in consumption order, so the first
    matmul can start early; dummy matmuls on a zeroed scratch warm the PE
    clock gate (HAM) while the first segments stream in.
  - PSUM -> SBUF evictions are single fused 128-partition copies (2 per pair,
    fp32 -> fp16 cast) so the eviction engines have ample slack vs the matmul
    cadence and never stall the TensorE (which would trip the HAM clock gate).
  - Output staged in fp16; fine-grained 9-row stores stream out in order on
    the sync DMA queue as soon as rows complete; host upcasts to fp32 and
    zeroes the crop window (no device memsets).
"""

import numpy as np

# ---- problem constants (hardcoded; kernel.py must be self-contained) ----
B, C, H, W = 16, 64, 128, 128
OC, KS = 64, 3
N_CORES = 8
IMGS = B // N_CORES  # 2 images per core

WP = W + 1            # padded row stride: 129
HP = H + 3            # padded rows in the x buffer: 131
XLEN = HP * WP        # 16899 fp32 per partition

RPC = 3               # output rows per chunk
NCH = (H + RPC - 1) // RPC          # 43 chunks per image (last has 2 rows)
NPAIR = 21            # chunk pairs (c, c+22); chunk 21 is the leftover
CHN = RPC * WP        # matmul free dim per full chunk: 387
STLEN = 2 * 66 * W    # stage free size: 16896 (= 132 rows * 128)

CROP0, CROP1 = 44, 84  # masked rows/cols [44, 84)

_CACHE = {}


def _build_module():
    import concourse.tile as tile
    from concourse import bacc, mybir

    f32 = mybir.dt.float32
    f16 = mybir.dt.float16
    bf16 = mybir.dt.bfloat16

    nc = bacc.Bacc("TRN2", target_bir_lowering=False, debug=False,
                   num_devices=N_CORES)

    x_ap = nc.dram_tensor("xin", [IMGS, C, XLEN], bf16,
                          kind="ExternalInput").ap()
    w_ap = nc.dram_tensor("wt", [C, KS * KS, OC], bf16,
                          kind="ExternalInput").ap()
    y_ap = nc.dram_tensor("yout", [IMGS, OC, H, W], f16,
                          kind="ExternalOutput").ap()

    x_bc = x_ap.rearrange("b c l -> (b c) l")  # [128, XLEN]

    with tile.TileContext(nc) as tc:
        with tc.tile_pool(name="big", bufs=1) as big, \
             tc.tile_pool(name="psum", bufs=8, space="PSUM") as pp:

            x_sb = big.tile([128, XLEN], bf16, tag="xbuf")
            stage = big.tile([128, STLEN], f16, tag="stage")
            w_sb = big.tile([128, KS * KS * OC], bf16, tag="wbuf")

            st3 = stage.rearrange("p (h w) -> p h w", w=W)    # [128, 132, 128]
            st4 = stage.rearrange("p (i h w) -> p i h w", i=2, w=W)

            # warm-up scratch for HAM (PE clock gate): dummy matmuls on zeros
            # keep the PE busy while x streams in, so real matmuls start at
            # the full 2.4 GHz clock instead of the cold 1.2 GHz.
            scr = big.tile([128, 512], bf16, tag="scratch")
            nc.vector.memset(scr[:, :], 0.0)

            # weights + x loads split across BOTH hardware DMA queues (sync +
            # scalar) so the first segments of the lower band (rows 0-65) and
            # upper band (rows 66-130) land concurrently; descriptor-issue
            # (~0.6us per dma_start) is also halved per queue.
            w_flat = w_ap.rearrange("i t o -> i (t o)")
            nc.sync.dma_start(out=w_sb[0:64, :], in_=w_flat)
            nc.scalar.dma_start(out=w_sb[64:128, :], in_=w_flat)
            for (a, b_) in [(0, 8), (8, 18), (18, 32), (32, 48), (48, 66)]:
                nc.sync.dma_start(out=x_sb[:, a * WP:b_ * WP],
                                  in_=x_bc[:, a * WP:b_ * WP])
            for (a, b_) in [(66, 74), (74, 84), (84, 98), (98, 114),
                            (114, 131)]:
                nc.scalar.dma_start(out=x_sb[:, a * WP:b_ * WP],
                                    in_=x_bc[:, a * WP:b_ * WP])

            def lhsT(half, t):
                return w_sb[half * 64:(half + 1) * 64, t * OC:(t + 1) * OC]

            def rhs(half, c, kh, kw, n):
                off = (RPC * c + kh) * WP + kw
                return x_sb[half * 64:(half + 1) * 64, off:off + n]

            TAPS = [(kh, kw) for kh in range(KS) for kw in range(KS)]

            # HAM warm-up: ~6 groups of 4-quadrant dummy matmuls (~2us cold)
            # issued before the real pairs; they only depend on the scratch
            # memset, so they run while the first x segments stream in.
            pdum1 = pp.tile([128, 512], f32, tag="ps")
            pdum2 = pp.tile([128, 512], f32, tag="ps")
            NDUM = 11
            for g in range(NDUM):
                st, sp = (g == 0), (g == NDUM - 1)
                for (pd_t, half) in ((pdum1, 0), (pdum2, 1)):
                    h0 = half * 64
                    lw = scr[h0:h0 + 64, 0:64]
                    rw = scr[h0:h0 + 64, 64:64 + CHN]
                    nc.tensor.matmul(pd_t[0:64, 0:CHN], lw, rw, start=st,
                                     stop=sp, skip_group_check=True)
                    nc.tensor.matmul(pd_t[64:128, 0:CHN], lw, rw, start=st,
                                     stop=sp, skip_group_check=True)

            def store(half, r0, nr, eng=None):
                # view rows [r0, r0+nr) of partition half -> y rows
                # 66*half + r0 ... for both images
                src = st4[half * 64:(half + 1) * 64, :, r0:r0 + nr, :]
                yr0 = 66 * half + r0
                dst = y_ap[:, :, yr0:yr0 + nr, :].rearrange(
                    "b o h w -> o b h w")
                (eng or nc.sync).dma_start(out=dst, in_=src)

            def do_pair(c):
                c2 = c + 22
                n2 = 2 * WP if c2 == NCH - 1 else CHN
                pa = pp.tile([128, 512], f32, tag="ps")
                pb = pp.tile([128, 512], f32, tag="ps")
                for t, (kh, kw) in enumerate(TAPS):
                    st, sp = (t == 0), (t == len(TAPS) - 1)
                    # img0 chunk c -> A[0:64];  img0 chunk c+22 -> A[64:128]
                    nc.tensor.matmul(pa[0:64, 0:CHN], lhsT(0, t),
                                     rhs(0, c, kh, kw, CHN), start=st, stop=sp,
                                     skip_group_check=True)
                    nc.tensor.matmul(pa[64:128, 0:n2], lhsT(0, t),
                                     rhs(0, c2, kh, kw, n2), start=st, stop=sp,
                                     skip_group_check=True)
                    # img1 chunk c -> B[0:64];  img1 chunk c+22 -> B[64:128]
                    nc.tensor.matmul(pb[0:64, 0:CHN], lhsT(1, t),
                                     rhs(1, c, kh, kw, CHN), start=st, stop=sp,
                                     skip_group_check=True)
                    nc.tensor.matmul(pb[64:128, 0:n2], lhsT(1, t),
                                     rhs(1, c2, kh, kw, n2), start=st, stop=sp,
                                     skip_group_check=True)

                # evict PSUM -> stage: one fused 128-partition copy per bank
                # (lower-half rows at free offset 3c, upper-half rows at the
                # same free offset on partitions 64-127).
                pa3 = pa[:, 0:CHN].rearrange("p (h w) -> p h w", w=WP)
                pb3 = pb[:, 0:CHN].rearrange("p (h w) -> p h w", w=WP)
                if c < NPAIR - 1:
                    nc.any.tensor_copy(st3[:, 3 * c:3 * c + 3, :],
                                       pa3[:, 0:3, 0:W])
                    nc.any.tensor_copy(st3[:, 66 + 3 * c:66 + 3 * c + 3, :],
                                       pb3[:, 0:3, 0:W])
                else:
                    # chunk 42 has only 2 rows on the upper half
                    nc.any.tensor_copy(st3[:, 60:62, :], pa3[:, 0:2, 0:W])
                    nc.any.tensor_copy(st3[0:64, 62:63, :],
                                       pa3[0:64, 2:3, 0:W])
                    nc.any.tensor_copy(st3[:, 126:128, :], pb3[:, 0:2, 0:W])
                    nc.any.tensor_copy(st3[0:64, 128:129, :],
                                       pb3[0:64, 2:3, 0:W])

            def do_leftover():
                # chunk 21 (lower rows 63-65), both images.  Computed on the
                # col-h64 PE pipes (psum partitions 64-127): the col-h0 pipes
                # carry 22 lower chunks vs 20.7 upper, so this rebalances the
                # critical path.  Results land in the unused upper-half stage
                # rows (img0: view i=0 rows 62-64, img1: i=1 rows 62-64) and
                # store via a dedicated per-image path.
                pc_ = pp.tile([128, 512], f32, tag="ps")
                pd_ = pp.tile([128, 512], f32, tag="ps")
                for t, (kh, kw) in enumerate(TAPS):
                    st, sp = (t == 0), (t == len(TAPS) - 1)
                    nc.tensor.matmul(pc_[64:128, 0:CHN], lhsT(0, t),
                                     rhs(0, 21, kh, kw, CHN), start=st,
                                     stop=sp, skip_group_check=True)
                    nc.tensor.matmul(pd_[64:128, 0:CHN], lhsT(1, t),
                                     rhs(1, 21, kh, kw, CHN), start=st,
                                     stop=sp, skip_group_check=True)
                pc3 = pc_[:, 0:CHN].rearrange("p (h w) -> p h w", w=WP)
                pd3 = pd_[:, 0:CHN].rearrange("p (h w) -> p h w", w=WP)
                nc.any.tensor_copy(st3[64:128, 62:65, :],
                                   pc3[64:128, 0:3, 0:W])
                nc.any.tensor_copy(st3[64:128, 128:131, :],
                                   pd3[64:128, 0:3, 0:W])

            def store_leftover():
                # y rows 63-65 per image from the upper-half stage slots
                for img, eng in ((0, nc.sync), (1, nc.scalar)):
                    src = st4[64:128, img:img + 1, 62:65, :]
                    dst = y_ap[img:img + 1, :, 63:66, :].rearrange(
                        "b o h w -> o b h w")
                    eng.dma_start(out=dst, in_=src)

            # stores fire every 3 pairs (9 output rows per half per image);
            # the leftover chunk runs mid-stream so only pair 20's small
            # stores remain after the last full pair.
            for c in range(16):
                do_pair(c)
                if c >= 2 and (c - 2) % 3 == 0:
                    k = (c - 2) // 3
                    store(0, 9 * k, 9)
                    store(1, 9 * k, 9)
            do_leftover()
            store_leftover()
            for c in range(16, NPAIR):
                do_pair(c)
                if c == 17:
                    store(0, 45, 9)
                    store(1, 45, 9)
                elif c >= 18:
                    # per-pair stores at the tail, split across both DMA
                    # queues, so the last transfer after the final matmul
                    # is tiny and issues in parallel
                    r0 = 54 + 3 * (c - 18)
                    store(0, r0, 3)
                    store(1, r0, 3 if c < 20 else 2, eng=nc.scalar)

    nc.compile()
    return nc


def _get_module():
    if "nc" not in _CACHE:
        _CACHE["nc"] = _build_module()
    return _CACHE["nc"]


def _make_in_maps(x, weight):
    x = np.asarray(x, dtype=np.float32)
    weight = np.asarray(weight, dtype=np.float32)
    # host marshaling: pad x into the row-major stride-129 layout
    xp = np.zeros((B, C, HP, WP), dtype=np.float32)
    xp[:, :, 1:H + 1, 1:W + 1] = x
    xp = xp.reshape(B, C, XLEN)
    import ml_dtypes
    xp = xp.astype(ml_dtypes.bfloat16)
    # weight [oc, ic, kh, kw] -> [ic, (kh kw), oc]
    wt = np.ascontiguousarray(
        weight.transpose(1, 2, 3, 0).reshape(C, KS * KS, OC)
    ).astype(ml_dtypes.bfloat16)
    return [
        {"xin": np.ascontiguousarray(xp[k * IMGS:(k + 1) * IMGS]), "wt": wt}
        for k in range(N_CORES)
    ]


def kernel(x, weight):
    from concourse.bass_utils import run_bass_kernel_spmd

    nc = _get_module()
    in_maps = _make_in_maps(x, weight)
    res = run_bass_kernel_spmd(nc, in_maps, list(range(N_CORES)))
    out = np.concatenate([res.results[k]["yout"] for k in range(N_CORES)],
                         axis=0).astype(np.float32)
    # crop mask applied on host (device never memsets the window)
    out[:, :, CROP0:CROP1, CROP0:CROP1] = 0.0
    return out


# revision 25
# speedup vs baseline: 1.2015x; 1.0151x over previous
"""Trainium2 Bass kernel for CropConv: 3x3 same-padding conv (64->64 ch) on
[16, 64, 128, 128] fp32 input, with a static crop mask zeroing output rows/cols
[44:84).

Strategy (data-parallel over batch, 8 cores x 2 images each):
  - Host marshals x into a zero-padded row-major layout with row stride 129
    (131 padded rows), so every conv tap (kh, kw) of an output row-chunk is one
    contiguous rhs slice.
  - Per core, image 0 lives in SBUF partitions 0-63 (partition = in-channel),
    image 1 in partitions 64-127.
  - The conv is 9 PSUM-accumulated TensorE matmuls per output chunk; four
    64x64-quadrant matmuls run concurrently (row-half = image, col-half =
    chunk pairing (c, c+22)).
  - x is loaded in 10 segments split across both hardware DMA queues
    (sync + scalar), lower/upper band in consumption order, so the first
    matmul can start early; dummy matmuls on a zeroed scratch warm the PE
    clock gate (HAM) while the first segments stream in.
  - PSUM -> SBUF evictions are single fused 128-partition copies (2 per pair,
    fp32 -> fp16 cast) so the eviction engines have ample slack vs the matmul
    cadence and never stall the TensorE (which would trip the HAM clock gate).
  - Output staged in fp16; fine-grained 9-row stores stream out in order on
    the sync DMA queue as soon as rows complete; host upcasts to fp32 and
    zeroes the crop window (no device memsets).
"""

import numpy as np

# ---- problem constants (hardcoded; kernel.py must be self-contained) ----
B, C, H, W = 16, 64, 128, 128
OC, KS = 64, 3
N_CORES = 8
IMGS = B // N_CORES  # 2 images per core

WP = W + 1            # padded row stride: 129
HP = H + 3            # padded rows in the x buffer: 131
XLEN = HP * WP        # 16899 fp32 per partition

RPC = 3               # output rows per chunk
NCH = (H + RPC - 1) // RPC          # 43 chunks per image (last has 2 rows)
NPAIR = 21            # chunk pairs (c, c+22); chunk 21 is the leftover
CHN = RPC * WP        # matmul free dim per full chunk: 387
STLEN = 2 * 66 * W    # stage free size: 16896 (= 132 rows * 128)

CROP0, CROP1 = 44, 84  # masked rows/cols [44, 84)

WOFF = KS * KS * OC   # weights ride in cols 0:576 of each x partition
XTOT = WOFF + XLEN    # total per-partition input length

_CACHE = {}


def _build_module():
    import concourse.tile as tile
    from concourse import bacc, mybir

    f32 = mybir.dt.float32
    f16 = mybir.dt.float16
    bf16 = mybir.dt.bfloat16

    nc = bacc.Bacc("TRN2", target_bir_lowering=False, debug=False,
                   num_devices=N_CORES)

    x_ap = nc.dram_tensor("xin", [IMGS, C, XTOT], bf16,
                          kind="ExternalInput").ap()
    y_ap = nc.dram_tensor("yout", [IMGS, OC, H, W], f16,
                          kind="ExternalOutput").ap()

    x_bc = x_ap.rearrange("b c l -> (b c) l")  # [128, XLEN]

    with tile.TileContext(nc) as tc:
        with tc.tile_pool(name="big", bufs=1) as big, \
             tc.tile_pool(name="psum", bufs=8, space="PSUM") as pp:

            x_sb = big.tile([128, XTOT], bf16, tag="xbuf")
            stage = big.tile([128, STLEN], f16, tag="stage")

            st3 = stage.rearrange("p (h w) -> p h w", w=W)    # [128, 132, 128]
            st4 = stage.rearrange("p (i h w) -> p i h w", i=2, w=W)

            # warm-up scratch for HAM (PE clock gate): dummy matmuls on zeros
            # keep the PE busy while x streams in, so real matmuls start at
            # the full 2.4 GHz clock instead of the cold 1.2 GHz.
            scr = big.tile([128, 512], bf16, tag="scratch")
            nc.vector.memset(scr[:, :], 0.0)

            # x loads split across BOTH hardware DMA queues (sync + scalar) so
            # the first segments of the lower band (rows 0-65) and upper band
            # (rows 66-130) land concurrently.  The weights ride in cols
            # 0:WOFF of the first lower segment (prepended per-partition by
            # the host) so they cost no extra DMA packets.
            for (a, b_) in [(0, 8), (8, 18), (18, 32), (32, 48), (48, 66)]:
                a_ = 0 if a == 0 else WOFF + a * WP
                nc.sync.dma_start(out=x_sb[:, a_:WOFF + b_ * WP],
                                  in_=x_bc[:, a_:WOFF + b_ * WP])
            for (a, b_) in [(66, 74), (74, 84), (84, 98), (98, 114),
                            (114, 131)]:
                nc.scalar.dma_start(
                    out=x_sb[:, WOFF + a * WP:WOFF + b_ * WP],
                    in_=x_bc[:, WOFF + a * WP:WOFF + b_ * WP])

            def lhsT(half, t):
                return x_sb[half * 64:(half + 1) * 64, t * OC:(t + 1) * OC]

            def rhs(half, c, kh, kw, n):
                off = WOFF + (RPC * c + kh) * WP + kw
                return x_sb[half * 64:(half + 1) * 64, off:off + n]

            TAPS = [(kh, kw) for kh in range(KS) for kw in range(KS)]

            # HAM warm-up: ~6 groups of 4-quadrant dummy matmuls (~2us cold)
            # issued before the real pairs; they only depend on the scratch
            # memset, so they run while the first x segments stream in.
            pdum1 = pp.tile([128, 512], f32, tag="ps")
            pdum2 = pp.tile([128, 512], f32, tag="ps")
            NDUM = 11
            for g in range(NDUM):
                st, sp = (g == 0), (g == NDUM - 1)
                for (pd_t, half) in ((pdum1, 0), (pdum2, 1)):
                    h0 = half * 64
                    lw = scr[h0:h0 + 64, 0:64]
                    rw = scr[h0:h0 + 64, 64:64 + CHN]
                    nc.tensor.matmul(pd_t[0:64, 0:CHN], lw, rw, start=st,
                                     stop=sp, skip_group_check=True)
                    nc.tensor.matmul(pd_t[64:128, 0:CHN], lw, rw, start=st,
                                     stop=sp, skip_group_check=True)

            def store(half, r0, nr, eng=None):
                # view rows [r0, r0+nr) of partition half -> y rows
                # 66*half + r0 ... for both images
                src = st4[half * 64:(half + 1) * 64, :, r0:r0 + nr, :]
                yr0 = 66 * half + r0
                dst = y_ap[:, :, yr0:yr0 + nr, :].rearrange(
                    "b o h w -> o b h w")
                (eng or nc.sync).dma_start(out=dst, in_=src)

            def do_pair(c):
                c2 = c + 22
                n2 = 2 * WP if c2 == NCH - 1 else CHN
                pa = pp.tile([128, 512], f32, tag="ps")
                pb = pp.tile([128, 512], f32, tag="ps")
                for t, (kh, kw) in enumerate(TAPS):
                    st, sp = (t == 0), (t == len(TAPS) - 1)
                    # img0 chunk c -> A[0:64];  img0 chunk c+22 -> A[64:128]
                    nc.tensor.matmul(pa[0:64, 0:CHN], lhsT(0, t),
                                     rhs(0, c, kh, kw, CHN), start=st, stop=sp,
                                     skip_group_check=True)
                    nc.tensor.matmul(pa[64:128, 0:n2], lhsT(0, t),
                                     rhs(0, c2, kh, kw, n2), start=st, stop=sp,
                                     skip_group_check=True)
                    # img1 chunk c -> B[0:64];  img1 chunk c+22 -> B[64:128]
                    nc.tensor.matmul(pb[0:64, 0:CHN], lhsT(1, t),
                                     rhs(1, c, kh, kw, CHN), start=st, stop=sp,
                                     skip_group_check=True)
                    nc.tensor.matmul(pb[64:128, 0:n2], lhsT(1, t),
                                     rhs(1, c2, kh, kw, n2), start=st, stop=sp,
                                     skip_group_check=True)

                # evict PSUM -> stage: one fused 128-partition copy per bank
                # (lower-half rows at free offset 3c, upper-half rows at the
                # same free offset on partitions 64-127).
                pa3 = pa[:, 0:CHN].rearrange("p (h w) -> p h w", w=WP)
                pb3 = pb[:, 0:CHN].rearrange("p (h w) -> p h w", w=WP)
                if c < NPAIR - 1:
                    nc.any.tensor_copy(st3[:, 3 * c:3 * c + 3, :],
                                       pa3[:, 0:3, 0:W])
                    nc.any.tensor_copy(st3[:, 66 + 3 * c:66 + 3 * c + 3, :],
                                       pb3[:, 0:3, 0:W])
                else:
                    # chunk 42 has only 2 rows on the upper half
                    nc.any.tensor_copy(st3[:, 60:62, :], pa3[:, 0:2, 0:W])
                    nc.any.tensor_copy(st3[0:64, 62:63, :],
                                       pa3[0:64, 2:3, 0:W])
                    nc.any.tensor_copy(st3[:, 126:128, :], pb3[:, 0:2, 0:W])
                    nc.any.tensor_copy(st3[0:64, 128:129, :],
                                       pb3[0:64, 2:3, 0:W])

            def do_leftover():
                # chunk 21 (lower rows 63-65), both images.  Computed on the
                # col-h64 PE pipes (psum partitions 64-127): the col-h0 pipes
                # carry 22 lower chunks vs 20.7 upper, so this rebalances the
                # critical path.  Results land in the unused upper-half stage
                # rows (img0: view i=0 rows 62-64, img1: i=1 rows 62-64) and
                # store via a dedicated per-image path.
                pc_ = pp.tile([128, 512], f32, tag="ps")
                pd_ = pp.tile([128, 512], f32, tag="ps")
                for t, (kh, kw) in enumerate(TAPS):
                    st, sp = (t == 0), (t == len(TAPS) - 1)
                    nc.tensor.matmul(pc_[64:128, 0:CHN], lhsT(0, t),
                                     rhs(0, 21, kh, kw, CHN), start=st,
                                     stop=sp, skip_group_check=True)
                    nc.tensor.matmul(pd_[64:128, 0:CHN], lhsT(1, t),
                                     rhs(1, 21, kh, kw, CHN), start=st,
                                     stop=sp, skip_group_check=True)
                pc3 = pc_[:, 0:CHN].rearrange("p (h w) -> p h w", w=WP)
                pd3 = pd_[:, 0:CHN].rearrange("p (h w) -> p h w", w=WP)
                nc.any.tensor_copy(st3[64:128, 62:65, :],
                                   pc3[64:128, 0:3, 0:W])
                nc.any.tensor_copy(st3[64:128, 128:131, :],
                                   pd3[64:128, 0:3, 0:W])

            def store_leftover():
                # y rows 63-65 per image from the upper-half stage slots
                for img, eng in ((0, nc.sync), (1, nc.scalar)):
                    src = st4[64:128, img:img + 1, 62:65, :]
                    dst = y_ap[img:img + 1, :, 63:66, :].rearrange(
                        "b o h w -> o b h w")
                    eng.dma_start(out=dst, in_=src)

            # stores fire every 3 pairs (9 output rows per half per image);
            # the leftover chunk runs mid-stream so only pair 20's small
            # stores remain after the last full pair.
            for c in range(16):
                do_pair(c)
                if c >= 2 and (c - 2) % 3 == 0:
                    k = (c - 2) // 3
                    store(0, 9 * k, 9)
                    store(1, 9 * k, 9)
            do_leftover()
            store_leftover()
            for c in range(16, NPAIR):
                do_pair(c)
                if c == 17:
                    store(0, 45, 9)
                    store(1, 45, 9)
                elif c >= 18:
                    # per-pair stores at the tail, split across both DMA
                    # queues, so the last transfer after the final matmul
                    # is tiny and issues in parallel
                    r0 = 54 + 3 * (c - 18)
                    store(0, r0, 3)
                    store(1, r0, 3 if c < 20 else 2, eng=nc.scalar)

    nc.compile()
    return nc


def _get_module():
    if "nc" not in _CACHE:
        _CACHE["nc"] = _build_module()
    return _CACHE["nc"]


def _make_in_maps(x, weight):
    x = np.asarray(x, dtype=np.float32)
    weight = np.asarray(weight, dtype=np.float32)
    # host marshaling: pad x into the row-major stride-129 layout, with the
    # weights ([oc, ic, kh, kw] -> [ic, (kh kw), oc] -> 576 values) prepended
    # to every partition so they ride in the first lower DMA segment
    xp = np.zeros((B, C, XTOT), dtype=np.float32)
    xpad = np.zeros((B, C, HP, WP), dtype=np.float32)
    xpad[:, :, 1:H + 1, 1:W + 1] = x
    xp[:, :, WOFF:] = xpad.reshape(B, C, XLEN)
    wt = np.ascontiguousarray(
        weight.transpose(1, 2, 3, 0).reshape(C, WOFF))
    xp[:, :, 0:WOFF] = wt[None, :, :]
    import ml_dtypes
    xp = xp.astype(ml_dtypes.bfloat16)
    return [
        {"xin": np.ascontiguousarray(xp[k * IMGS:(k + 1) * IMGS])}
        for k in range(N_CORES)
    ]


def kernel(x, weight):
    from concourse.bass_utils import run_bass_kernel_spmd

    nc = _get_module()
    in_maps = _make_in_maps(x, weight)
    res = run_bass_kernel_spmd(nc, in_maps, list(range(N_CORES)))
    out = np.concatenate([res.results[k]["yout"] for k in range(N_CORES)],
                         axis=0).astype(np.float32)
    # crop mask applied on host (device never memsets the window)
    out[:, :, CROP0:CROP1, CROP0:CROP1] = 0.0
    return out
